# revision 1
# baseline (speedup 1.0000x reference)
"""Trainium2 Bass kernel for nn_CustomLoss (gnn_message_passing).

Computes, SPMD over 8 NeuronCores:
  loss = ||a - p||_F + lamb*(||relu(W)||_F + ||relu(E)||_F)
         + sum_g diff_w[g] * sum_m Sw[j_g, i_gm]
         + diff_e * sum(Se[row, e_j])

Sharding (hardcoded, matches the problem's full shapes):
  - actual/prediction row-sharded 512 rows/core (the dominant 256 MB stream)
  - group dim G sharded 128 groups/core; W-column gathers for each group
    shard are routed host-side to the owning core (index routing only,
    all arithmetic on device)
  - relu penalties sharded (W by columns, E by rows)
  - entity term replicated (tiny); core 0's value is used
  - per-core scalar partials combined on host (8x6 values + 3 sqrts)
"""

import ml_dtypes
import numpy as np

import concourse.bass as bass
from concourse import mybir
from concourse.bass_utils import run_bass_kernel_spmd

NC = 8
N_E, N_W, K = 4096, 8192, 128
G, M, J = 1024, 64, 256
GS = G // NC            # 128 groups per core
RS = N_E // NC          # 512 rows of actual/prediction per core
CH = 4096               # free-dim chunk for the big stream
NRT = RS // 128         # 4 row tiles per core
NCC = N_W // CH         # 2 col chunks
NCHUNK = NRT * NCC      # 8 chunks per tensor per core
KC = 2                  # wi processed in KC chunks of [128, K//KC * M]
WSH = N_W // NC         # 1024 W columns per core (relu penalty shard)
ESH = (N_E // NC) * K // 128   # 512: E rows per core laid out [128, 512]
JB = J // 128           # 2 entity blocks

# packed fp32 small inputs: wj | swg | sev
O_WJ = 0
O_SWG = O_WJ + K
O_SEV = O_SWG + M
SM_TOT = O_SEV + JB
# packed bf16 small inputs (terms insensitive to rounding): wsh | esh | ej | ei
H_WSH = 0
H_ESH = H_WSH + WSH
H_EJ = H_ESH + ESH
H_EI = H_EJ + JB * K
SMH_TOT = H_EI + JB * K

f32 = mybir.dt.float32
bf16 = mybir.dt.bfloat16

_CACHE = {}
LAST_RESULTS = None     # BassKernelResults of the most recent run (for profiling)


def _build_module():
    """Raw-bass pipeline with explicit semaphores.

    All cross-engine waits are standalone wait_ge instructions (never more
    than one sync-wait on any DMA/compute instruction — walrus's per-ISA
    wait-slot limits reject the schedules Tile generates for this pattern).
    """
    from contextlib import ExitStack

    nc = bass.Bass()

    ap_d = nc.dram_tensor("ap", [NRT, 128, 2, N_W], f32, kind="ExternalInput")
    wi_d = nc.dram_tensor("wi", [128, K * M], bf16, kind="ExternalInput")
    sm_d = nc.dram_tensor("sm", [128, SM_TOT], f32, kind="ExternalInput")
    smh_d = nc.dram_tensor("smh", [128, SMH_TOT], bf16, kind="ExternalInput")
    out_d = nc.dram_tensor("out", [1, 8], f32, kind="ExternalOutput")

    SUB = mybir.AluOpType.subtract
    SQUARE = mybir.ActivationFunctionType.Square
    SQRT = mybir.ActivationFunctionType.Sqrt
    X = mybir.AxisListType.X
    KH = K // KC
    NB = 3                      # apt ring depth

    ctx = ExitStack()
    apt = [ctx.enter_context(nc.sbuf_tensor(f"apt{i}", [128, 2, CH], f32)) for i in range(NB)]
    dbuf = [ctx.enter_context(nc.sbuf_tensor(f"dbuf{i}", [128, CH], f32)) for i in range(2)]
    wibuf = ctx.enter_context(nc.sbuf_tensor("wibuf", [128, K * M], bf16))
    smbuf = ctx.enter_context(nc.sbuf_tensor("smbuf", [128, SM_TOT], f32))
    smhbuf = ctx.enter_context(nc.sbuf_tensor("smhbuf", [128, SMH_TOT], bf16))
    dwbuf = ctx.enter_context(nc.sbuf_tensor("dwbuf", [128, (K // KC) * M], f32))
    wshs = ctx.enter_context(nc.sbuf_tensor("wshs", [128, WSH], f32))
    eshs = ctx.enter_context(nc.sbuf_tensor("eshs", [128, ESH], f32))
    det = ctx.enter_context(nc.sbuf_tensor("det", [128, JB * K], f32))
    parts = ctx.enter_context(nc.sbuf_tensor("parts", [128, 6], f32))
    rparts = ctx.enter_context(nc.sbuf_tensor("rparts", [128, 2 * NCHUNK + 2], f32))
    wparts = ctx.enter_context(nc.sbuf_tensor("wparts", [128, KC], f32))
    ones = ctx.enter_context(nc.sbuf_tensor("ones", [128, 1], f32))
    diff2 = ctx.enter_context(nc.sbuf_tensor("diff2", [128, 1], f32))
    diffw = ctx.enter_context(nc.sbuf_tensor("diffw", [128, 1], f32))
    swsum = ctx.enter_context(nc.sbuf_tensor("swsum", [128, 1], f32))
    ot = ctx.enter_context(nc.sbuf_tensor("ot", [1, 8], f32))
    esq = ctx.enter_context(nc.sbuf_tensor("esq", [1, 1], f32))
    psum = ctx.enter_context(nc.psum_tensor("psumt", [1, 6], f32))

    s_dsm = ctx.enter_context(nc.semaphore("s_dsm"))
    # per-slot semaphores for the apt ring: each round adds 16 (DMA done)
    # + 1 (DVE consumed) = 17, so one threshold covers WAW + WAR
    s_slot = [ctx.enter_context(nc.semaphore(f"s_slot{b}")) for b in range(NB)]
    s_sub = ctx.enter_context(nc.semaphore("s_sub"))
    s_bsq = ctx.enter_context(nc.semaphore("s_bsq"))
    s_wsub = ctx.enter_context(nc.semaphore("s_wsub"))
    s_wsq = ctx.enter_context(nc.semaphore("s_wsq"))
    s_d2 = ctx.enter_context(nc.semaphore("s_d2"))
    s_sqr = ctx.enter_context(nc.semaphore("s_sqr"))
    s_esub = ctx.enter_context(nc.semaphore("s_esub"))
    s_parts = ctx.enter_context(nc.semaphore("s_parts"))
    s_pe = ctx.enter_context(nc.semaphore("s_pe"))
    s_esq = ctx.enter_context(nc.semaphore("s_esq"))
    s_fin = ctx.enter_context(nc.semaphore("s_fin"))
    s_last = [ctx.enter_context(nc.semaphore(f"s_last{q}")) for q in range(3)]
    s_dout = ctx.enter_context(nc.semaphore("s_dout"))

    def wi_view(c):
        return wibuf[:, c * KH * M:(c + 1) * KH * M].rearrange(
            "g (k m) -> g k m", m=M)

    def wj_bcast(c):
        sl = smbuf[:, O_WJ + c * KH:O_WJ + (c + 1) * KH]
        return bass.AP(tensor=sl.tensor, offset=sl.offset, ap=[*sl.ap, [0, M]])

    def dw_view():
        return dwbuf[:].rearrange("g (k m) -> g k m", m=M)

    with ctx, nc.Block(no_gpsimd_drain=True) as block:

        LAST = NCHUNK - 1
        HW2 = CH // 2

        @block.sync
        def _(sync):
            sync.dma_start(out=smbuf[:], in_=sm_d[:, :]).then_inc(s_dsm, 16)
            sync.dma_start(out=smhbuf[:], in_=smh_d[:, :]).then_inc(s_dsm, 16)
            sync.dma_start(out=wibuf[:], in_=wi_d[:, :]).then_inc(s_dsm, 16)
            for i in range(NCHUNK):
                t, j = divmod(i, NCC)
                b, k = i % NB, i // NB
                if k > 0:
                    sync.wait_ge(s_slot[b], 17 * k)
                if i == LAST:
                    # split the final chunk into four 1MB sub-DMAs so the
                    # end-of-stream compute tail is one quarter, not a half
                    Q = CH // 4
                    for q in range(4):
                        sem = s_slot[b] if q == 0 else s_last[q - 1]
                        sync.dma_start(
                            out=apt[b][:, :, q * Q:(q + 1) * Q],
                            in_=ap_d[t, :, :, j * CH + q * Q:j * CH + (q + 1) * Q],
                        ).then_inc(sem, 16)
                else:
                    sync.dma_start(
                        out=apt[b][:],
                        in_=ap_d[t, :, :, j * CH:(j + 1) * CH],
                    ).then_inc(s_slot[b], 16)
            sync.wait_ge(s_fin, 1)
            sync.dma_start(out=out_d[:, :], in_=ot[:, :]).then_inc(s_dout, 16)
            sync.wait_ge(s_dout, 16)

        @block.vector
        def _(v):
            v.memset(ones[:], 1.0)
            v.memset(ot[:], 0.0)
            v.wait_ge(s_dsm, 48)
            # word chunk 0
            v.tensor_tensor(out=dw_view(), in0=wi_view(0), in1=wj_bcast(0),
                            op=SUB).then_inc(s_wsub, 1)
            # entity subtract
            v.tensor_tensor(out=det[:], in0=smhbuf[:, H_EJ:H_EJ + JB * K],
                            in1=smhbuf[:, H_EI:H_EI + JB * K],
                            op=SUB).then_inc(s_esub, 1)
            # Se row sum
            v.reduce_sum(parts[:, 5:6], smbuf[:, O_SEV:O_SEV + JB],
                         axis=X).then_inc(s_parts, 1)
            # relu penalties
            v.scalar_tensor_tensor(
                out=wshs[:], in0=smhbuf[:, H_WSH:H_WSH + WSH], scalar=0.0,
                in1=smhbuf[:, H_WSH:H_WSH + WSH], op0=mybir.AluOpType.max,
                op1=mybir.AluOpType.mult,
                accum_out=parts[:, 1:2]).then_inc(s_parts, 1)
            v.scalar_tensor_tensor(
                out=eshs[:], in0=smhbuf[:, H_ESH:H_ESH + ESH], scalar=0.0,
                in1=smhbuf[:, H_ESH:H_ESH + ESH], op0=mybir.AluOpType.max,
                op1=mybir.AluOpType.mult,
                accum_out=parts[:, 2:3]).then_inc(s_parts, 1)
            v.reduce_sum(swsum[:], smbuf[:, O_SWG:O_SWG + M], axis=X)
            # word chunk 1 (dwbuf freed once ACT squared chunk 0)
            v.wait_ge(s_wsq, 1)
            v.tensor_tensor(out=dw_view(), in0=wi_view(1), in1=wj_bcast(1),
                            op=SUB).then_inc(s_wsub, 1)
            v.wait_ge(s_wsq, 2)
            v.reduce_sum(diff2[:], wparts[:], axis=X).then_inc(s_d2, 1)
            v.wait_ge(s_sqr, 1)
            v.tensor_mul(parts[:, 3:4], diffw[:], swsum[:]).then_inc(s_parts, 1)
            # big stream: DMA chunks of CH, computed in CH/2 halves so the
            # ScalarE square of half 0 overlaps the subtract of half 1
            H = CH // 2
            for i in range(NCHUNK):
                b, k = i % NB, i // NB
                v.wait_ge(s_slot[b], 17 * k + 16)
                nparts = 2 if i < NCHUNK - 1 else 4
                P = CH // nparts
                for c in range(nparts):
                    h = 2 * i + c
                    if i == NCHUNK - 1 and c > 0:
                        v.wait_ge(s_last[c - 1], 16)
                    if h >= 2:
                        v.wait_ge(s_bsq, h - 1)
                    last_piece = c == nparts - 1
                    sem = s_slot[b] if last_piece else s_sub
                    v.tensor_tensor(
                        out=dbuf[h % 2][:, :P],
                        in0=apt[b][:, 0, c * P:(c + 1) * P],
                        in1=apt[b][:, 1, c * P:(c + 1) * P],
                        op=SUB).then_inc(sem, 1)
            v.wait_ge(s_bsq, 2 * NCHUNK + 2)
            v.reduce_sum(parts[:, 0:1], rparts[:], axis=X).then_inc(s_parts, 1)
            # final assembly
            v.wait_ge(s_pe, 1)
            v.tensor_copy(ot[0:1, 0:4], psum[0:1, 0:4])
            v.wait_ge(s_esq, 1)
            v.tensor_mul(ot[0:1, 4:5], esq[:], psum[0:1, 5:6]).then_inc(s_fin, 1)

        @block.scalar
        def _(a):
            a.wait_ge(s_wsub, 1)
            a.activation(out=dwbuf[:], in_=dwbuf[:], func=SQUARE,
                         accum_out=wparts[:, 0:1]).then_inc(s_wsq, 1)
            a.wait_ge(s_esub, 1)
            a.activation(out=det[:], in_=det[:], func=SQUARE,
                         accum_out=parts[:, 4:5]).then_inc(s_parts, 1)
            a.wait_ge(s_wsub, 2)
            a.activation(out=dwbuf[:], in_=dwbuf[:], func=SQUARE,
                         accum_out=wparts[:, 1:2]).then_inc(s_wsq, 1)
            a.wait_ge(s_d2, 1)
            a.activation(out=diffw[:], in_=diff2[:], func=SQRT).then_inc(s_sqr, 1)
            nsub = 0
            for i in range(NCHUNK):
                b, k = i % NB, i // NB
                nparts = 2 if i < NCHUNK - 1 else 4
                P = CH // nparts
                for c in range(nparts):
                    h = 2 * i + c
                    if c == nparts - 1:
                        a.wait_ge(s_slot[b], 17 * k + 17)
                    else:
                        nsub += 1
                        a.wait_ge(s_sub, nsub)
                    a.activation(out=dbuf[h % 2][:, :P], in_=dbuf[h % 2][:, :P],
                                 func=SQUARE,
                                 accum_out=rparts[:, h:h + 1]).then_inc(s_bsq, 1)
            a.wait_ge(s_pe, 1)
            a.activation(out=esq[:], in_=psum[0:1, 4:5],
                         func=SQRT).then_inc(s_esq, 1)

        @block.tensor
        def _(t):
            t.wait_ge(s_parts, 6)
            nc.tensor.matmul(out=psum[:], lhsT=ones[:], rhs=parts[:],
                             start=True, stop=True).then_inc(s_pe, 1)

    return nc


def _shard_inputs(inputs):
    actual = np.ascontiguousarray(np.asarray(inputs["actual"], dtype=np.float32))
    prediction = np.ascontiguousarray(np.asarray(inputs["prediction"], dtype=np.float32))
    W = np.asarray(inputs["W"], dtype=np.float32)
    E = np.asarray(inputs["E"], dtype=np.float32)
    Sw = np.asarray(inputs["Sw"], dtype=np.float32)
    Se = inputs["Se"]
    row_ind = int(inputs["row_ind"])
    word_i = np.asarray(inputs["word_i_indices"], dtype=np.int64)
    entity_j = np.asarray(inputs["entity_j_indices"], dtype=np.int64)
    sample_j = np.asarray(inputs["sample_j_indices"], dtype=np.int64)

    # entity term data (replicated on all cores)
    ej_h = np.asarray(E[entity_j]).reshape(JB, 128, K).transpose(1, 0, 2).reshape(128, JB * K)
    ei_h = np.tile(np.asarray(E[row_ind]), (128, JB))
    sev_h = np.asarray(Se[row_ind])[entity_j].reshape(JB, 128).T.astype(np.float32)

    in_maps = []
    for c in range(NC):
        gsl = slice(c * GS, (c + 1) * GS)
        idx = word_i[gsl]                       # [GS, M]
        sj = sample_j[gsl]                      # [GS]
        wi_h = np.ascontiguousarray(
            W[:, idx].transpose(1, 0, 2).reshape(GS, K * M)
        ).astype(ml_dtypes.bfloat16)
        sm = np.empty((128, SM_TOT), dtype=np.float32)
        sm[:, O_WJ:O_WJ + K] = W[:, sj].T
        sm[:, O_SWG:O_SWG + M] = Sw[sj[:, None], idx]
        sm[:, O_SEV:O_SEV + JB] = sev_h
        smh = np.empty((128, SMH_TOT), dtype=ml_dtypes.bfloat16)
        smh[:, H_WSH:H_WSH + WSH] = W[:, c * WSH:(c + 1) * WSH]
        smh[:, H_ESH:H_ESH + ESH] = (
            E[c * RS:(c + 1) * RS].reshape(NRT, 128, K)
            .transpose(1, 0, 2).reshape(128, NRT * K))
        smh[:, H_EJ:H_EJ + JB * K] = ej_h
        smh[:, H_EI:H_EI + JB * K] = ei_h
        ap = np.empty((NRT, 128, 2, N_W), dtype=np.float32)
        ap[:, :, 0, :] = actual[c * RS:(c + 1) * RS].reshape(NRT, 128, N_W)
        ap[:, :, 1, :] = prediction[c * RS:(c + 1) * RS].reshape(NRT, 128, N_W)
        in_maps.append({
            "ap": ap,
            "wi": wi_h,
            "sm": sm,
            "smh": smh,
        })
    return in_maps


def kernel(**inputs):
    global LAST_RESULTS
    import os

    if "nc" not in _CACHE:
        _CACHE["nc"] = _build_module()
    nc = _CACHE["nc"]

    in_maps = _shard_inputs(inputs)
    trace = bool(int(os.environ.get("KERNEL_TRACE", "0")))
    res = run_bass_kernel_spmd(nc, in_maps, list(range(NC)), trace=trace)
    LAST_RESULTS = res

    sums = np.stack([np.asarray(r["out"], dtype=np.float64)[0]
                     for r in res.results])          # [NC, 8]
    recon = np.sqrt(sums[:, 0].sum())
    relu_w = np.sqrt(sums[:, 1].sum())
    relu_e = np.sqrt(sums[:, 2].sum())
    word = sums[:, 3].sum()
    ent = sums[0, 4]
    lamb = float(np.asarray(inputs["lamb"]))
    total = recon + lamb * (relu_w + relu_e) + word + ent
    return np.asarray(total, dtype=np.float32)



# revision 3
# speedup vs baseline: 1.8935x; 1.8935x over previous
"""Trainium2 Bass kernel for nn_CustomLoss (gnn_message_passing).

Computes, SPMD over 8 NeuronCores:
  loss = ||a - p||_F + lamb*(||relu(W)||_F + ||relu(E)||_F)
         + sum_g diff_w[g] * sum_m Sw[j_g, i_gm]
         + diff_e * sum(Se[row, e_j])

v2 design (vs the fp32-stream v1):
  - actual/prediction stream in fp8 (e4m3), planar pairs [128, 2, cols].
    The loss total (~4.2e6) is dominated by the word-similarity term;
    recon (~8e3) tolerates fp8 rounding (bias ~6e-4 on recon, ~1e-6 on
    the total).
  - The TensorE computes d = a - p via DoubleRow fp8 matmuls with a
    [+I; -I] stationary, writing full [128, 1024] psum tiles.
  - Stream squares: ACT (activation Square + accum, direct from psum)
    and DVE (bn_stats, 2x512 per tile; host recovers sum-of-squares
    from count/mean/M2). Tiles are assigned statically; each psum bank
    is read by exactly one engine (concurrent same-bank reads from two
    engines fault the HW).
  - Word term: wi gathered host-side (index routing), shipped fp8;
    Pool does dw = wj_bcast - wi (f32 out), ACT square-accums per
    group partition. Host does sqrt(dw2)*swsum.
  - All final reductions/sqrts on host from a [128, 240] partial
    tensor per core.
"""

import ml_dtypes
import numpy as np

import concourse.bass as bass
from concourse import mybir
from concourse.bass_utils import run_bass_kernel_spmd

NC = 8
N_E, N_W, K = 4096, 8192, 128
G, M, J = 1024, 64, 256
GS = G // NC              # 128 groups per core
RS = N_E // NC            # 512 rows of actual/prediction per core
NRT = RS // 128           # 4 row tiles per core

NCHUNK = 16               # x chunks per core
CCOL = 2048               # d-cols per chunk
NTILE = 32                # psum tiles (1024 cols each)
TCOL = 1024
NRING = 4                 # x ring slots
NPS = 4                   # psum slots

WIC = 8                   # wi processed in 8 chunks of 1024
WICOL = 1024
KPC = WICOL // M          # 16 k's per wi chunk

WSH = N_W // NC           # 1024 W columns per core (relu shard)
ESH = RS * K // 128       # 512 E values per partition (relu shard)
JB = J // 128             # 2 entity blocks

# smh (fp8) layout: wsh | esh | ej | ei
H_WSH = 0
H_ESH = H_WSH + WSH
H_EJ = H_ESH + ESH
H_EI = H_EJ + JB * K
SMH_TOT = H_EI + JB * K
# smf (f32) layout: swg | sev | wj
F_SWG = 0
F_SEV = F_SWG + M
F_WJ = F_SEV + JB
SMF_TOT = F_WJ + K

# outbuf (f32) column layout
O_ASQ = 0                  # 32: ACT stream-tile accums (only ACT tiles valid)
O_BNS = 32                 # 15*12: DVE bns stats, compact per DVE tile
O_WI = O_BNS + 15 * 12     # 8: wi-sq chunk accums (per-partition = per-group)
O_RELW = O_WI + WIC        # 1
O_RELE = O_RELW + 1        # 1
O_ENT = O_RELE + 1         # 1
O_SWS = O_ENT + 1          # 1
O_SEVS = O_SWS + 1         # 1
OUT_TOT = 240

f32 = mybir.dt.float32
f8 = mybir.dt.float8e4

SUB = mybir.AluOpType.subtract
MULT = mybir.AluOpType.mult
MAX = mybir.AluOpType.max
ADD = mybir.AluOpType.add
X = mybir.AxisListType.X
SQUARE = mybir.ActivationFunctionType.Square

# stream tile -> engine ('A' = ACT apsq, 'D' = DVE bn_stats)
TILE_ENG = ['A' if t % 2 == 0 else 'D' for t in range(NTILE)]
TILE_ENG[31] = 'A'  # ACT 17, DVE 15
ACT_TILES = [t for t in range(NTILE) if TILE_ENG[t] == 'A']
DVE_TILES = [t for t in range(NTILE) if TILE_ENG[t] == 'D']
# positions (in ACT's own stream-tile sequence) after which to insert
# wi-sq chunks 0..7 (pool chunk j ready ~ 8 + 2.1*j us)
ACT_WI_POS = {6: 0, 8: 1, 10: 2, 12: 3, 13: 4, 14: 5, 15: 6, 16: 7}
# positions in DVE's sequence for small ops
DVE_RELW_POS = 5
DVE_RELE_POS = 6
DVE_ENT_POS = 7
DVE_SUMS_POS = 4

_CACHE = {}
LAST_RESULTS = None


def _build_module():
    from contextlib import ExitStack

    nc = bass.Bass()

    x_d = nc.dram_tensor("x", [NCHUNK, 128, 2, CCOL], f8, kind="ExternalInput")
    wi_d = nc.dram_tensor("wi", [128, K * M], f8, kind="ExternalInput")
    id_d = nc.dram_tensor("ident", [128, 256], f8, kind="ExternalInput")
    smh_d = nc.dram_tensor("smh", [128, SMH_TOT], f8, kind="ExternalInput")
    smf_d = nc.dram_tensor("smf", [128, SMF_TOT], f32, kind="ExternalInput")
    out_d = nc.dram_tensor("out", [128, OUT_TOT], f32, kind="ExternalOutput")

    ctx = ExitStack()
    xb = [ctx.enter_context(nc.sbuf_tensor(f"xb{b}", [128, 2, CCOL], f8))
          for b in range(NRING)]
    wib = ctx.enter_context(nc.sbuf_tensor("wib", [128, K * M], f8))
    idb = ctx.enter_context(nc.sbuf_tensor("idb", [128, 256], f8))
    smhb = ctx.enter_context(nc.sbuf_tensor("smhb", [128, SMH_TOT], f8))
    smfb = ctx.enter_context(nc.sbuf_tensor("smfb", [128, SMF_TOT], f32))
    dwb = ctx.enter_context(nc.sbuf_tensor("dwb", [128, K * M], f32))
    ob = ctx.enter_context(nc.sbuf_tensor("ob", [128, OUT_TOT], f32))
    ascr = ctx.enter_context(nc.sbuf_tensor("ascr", [128, TCOL], f32))
    awscr = ctx.enter_context(nc.sbuf_tensor("awscr", [128, WICOL], f32))
    vscr = ctx.enter_context(nc.sbuf_tensor("vscr", [128, WSH], f32))
    detb = ctx.enter_context(nc.sbuf_tensor("detb", [128, JB * K], f32))
    escr = ctx.enter_context(nc.sbuf_tensor("escr", [128, JB * K], f32))
    ps = [ctx.enter_context(nc.psum_tensor(f"ps{i}", [128, TCOL], f32))
          for i in range(NPS)]

    s_id = ctx.enter_context(nc.semaphore("s_id"))
    s_wi = ctx.enter_context(nc.semaphore("s_wi"))
    s_smh = ctx.enter_context(nc.semaphore("s_smh"))
    s_smf = ctx.enter_context(nc.semaphore("s_smf"))
    s_xd = [ctx.enter_context(nc.semaphore(f"s_xd{b}")) for b in range(NRING)]
    s_mm = ctx.enter_context(nc.semaphore("s_mm"))
    s_psf = [ctx.enter_context(nc.semaphore(f"s_psf{i}")) for i in range(NPS)]
    s_dw = ctx.enter_context(nc.semaphore("s_dw"))
    s_ent = ctx.enter_context(nc.semaphore("s_ent"))
    s_done = ctx.enter_context(nc.semaphore("s_done"))
    s_out = ctx.enter_context(nc.semaphore("s_out"))

    def wj_bcast(j):
        # [128, KPC] slice of wj, broadcast M times along free (k outer, m inner)
        sl = smfb[:, F_WJ + j * KPC:F_WJ + (j + 1) * KPC]
        return bass.AP(tensor=sl.tensor, offset=sl.offset, ap=[*sl.ap, [0, M]])

    with ctx, nc.Block(no_gpsimd_drain=True) as block:

        @block.sync
        def _(sync):
            sync.dma_start(out=idb[:], in_=id_d[:, :]).then_inc(s_id, 16)
            sync.dma_start(out=xb[0][:], in_=x_d[0, :, :, :]).then_inc(s_xd[0], 16)
            sync.dma_start(out=xb[1][:], in_=x_d[1, :, :, :]).then_inc(s_xd[1], 16)
            sync.dma_start(out=smfb[:], in_=smf_d[:, :]).then_inc(s_smf, 16)
            sync.dma_start(out=wib[:], in_=wi_d[:, :]).then_inc(s_wi, 16)
            sync.dma_start(out=xb[2][:], in_=x_d[2, :, :, :]).then_inc(s_xd[2], 16)
            sync.dma_start(out=smhb[:], in_=smh_d[:, :]).then_inc(s_smh, 16)
            sync.dma_start(out=xb[3][:], in_=x_d[3, :, :, :]).then_inc(s_xd[3], 16)
            for c in range(NRING, NCHUNK):
                # ring slot reusable once both tiles of chunk c-NRING are matmul'd
                sync.wait_ge(s_mm, 2 * (c - NRING) + 2)
                sync.dma_start(out=xb[c % NRING][:],
                               in_=x_d[c, :, :, :]).then_inc(s_xd[c % NRING], 16)
            sync.wait_ge(s_done, 2)
            sync.dma_start(out=out_d[:, :], in_=ob[:]).then_inc(s_out, 16)
            sync.wait_ge(s_out, 16)

        @block.tensor
        def _(t):
            t.wait_ge(s_id, 16)
            lhsT = idb[:].rearrange("p (two m) -> p two m", two=2)
            for ti in range(NTILE):
                c, s, b = ti // 2, ti % NPS, (ti // 2) % NRING
                if ti % 2 == 0:
                    t.wait_ge(s_xd[b], 16 * (c // NRING + 1))
                if ti >= NPS:
                    t.wait_ge(s_psf[s], ti // NPS)
                for h in range(2):
                    col = (ti % 2) * TCOL + h * 512
                    mm = nc.tensor.matmul(
                        out=ps[s][:, h * 512:(h + 1) * 512],
                        lhsT=lhsT,
                        rhs=xb[b][:, :, col:col + 512],
                        start=True, stop=True,
                        perf_mode=mybir.MatmulPerfMode.DoubleRow,
                    )
                    if h == 1:
                        mm.then_inc(s_mm, 1)

        @block.scalar
        def _(a):
            nseq = 0
            for i, ti in enumerate(ACT_TILES):
                s = ti % NPS
                a.wait_ge(s_mm, ti + 1)
                a.activation(out=ascr[:], in_=ps[s][:],
                             func=SQUARE,
                             accum_out=ob[:, O_ASQ + ti:O_ASQ + ti + 1]
                             ).then_inc(s_psf[s], 1)
                nseq += 1
                if i == 9:
                    # entity: square det (after DVE sub, smh-dependent)
                    a.wait_ge(s_ent, 1)
                    a.activation(out=escr[:], in_=detb[:], func=SQUARE,
                                 accum_out=ob[:, O_ENT:O_ENT + 1])
                if i in ACT_WI_POS:
                    j = ACT_WI_POS[i]
                    a.wait_ge(s_dw, j + 1)
                    a.activation(out=awscr[:],
                                 in_=dwb[:, j * WICOL:(j + 1) * WICOL],
                                 func=SQUARE,
                                 accum_out=ob[:, O_WI + j:O_WI + j + 1])
            a.copy(ob[:, O_ASQ:O_ASQ + 1], ob[:, O_ASQ:O_ASQ + 1]).then_inc(s_done, 1)

        @block.vector
        def _(v):
            for i, ti in enumerate(DVE_TILES):
                s = ti % NPS
                v.wait_ge(s_mm, ti + 1)
                v.bn_stats(out=ob[:, O_BNS + i * 12:O_BNS + i * 12 + 6],
                           in_=ps[s][:, 0:512])
                v.bn_stats(out=ob[:, O_BNS + i * 12 + 6:O_BNS + i * 12 + 12],
                           in_=ps[s][:, 512:TCOL]).then_inc(s_psf[s], 1)
                if i == DVE_SUMS_POS:
                    v.wait_ge(s_smf, 16)
                    v.reduce_sum(ob[:, O_SWS:O_SWS + 1],
                                 smfb[:, F_SWG:F_SWG + M], axis=X)
                    v.reduce_sum(ob[:, O_SEVS:O_SEVS + 1],
                                 smfb[:, F_SEV:F_SEV + JB], axis=X)
                if i == DVE_RELW_POS:
                    v.wait_ge(s_smh, 16)
                    v.scalar_tensor_tensor(
                        out=vscr[:, 0:WSH], in0=smhb[:, H_WSH:H_WSH + WSH],
                        scalar=0.0, in1=smhb[:, H_WSH:H_WSH + WSH],
                        op0=MAX, op1=MULT,
                        accum_out=ob[:, O_RELW:O_RELW + 1])
                if i == DVE_RELE_POS:
                    v.scalar_tensor_tensor(
                        out=vscr[:, 0:ESH], in0=smhb[:, H_ESH:H_ESH + ESH],
                        scalar=0.0, in1=smhb[:, H_ESH:H_ESH + ESH],
                        op0=MAX, op1=MULT,
                        accum_out=ob[:, O_RELE:O_RELE + 1])
                if i == DVE_ENT_POS:
                    v.tensor_tensor(out=detb[:],
                                    in0=smhb[:, H_EJ:H_EJ + JB * K],
                                    in1=smhb[:, H_EI:H_EI + JB * K],
                                    op=SUB).then_inc(s_ent, 1)
            v.engine_nop().then_inc(s_done, 1)

        @block.gpsimd
        def _(g):
            g.wait_ge(s_wi, 16)
            g.wait_ge(s_smf, 16)
            for j in range(WIC):
                g.tensor_tensor(
                    out=dwb[:, j * WICOL:(j + 1) * WICOL],
                    in0=wj_bcast(j),
                    in1=wib[:, j * WICOL:(j + 1) * WICOL],
                    op=SUB).then_inc(s_dw, 1)

    return nc


def _shard_inputs(inputs):
    fp8 = ml_dtypes.float8_e4m3
    actual = np.asarray(inputs["actual"], dtype=np.float32)
    prediction = np.asarray(inputs["prediction"], dtype=np.float32)
    W = np.asarray(inputs["W"], dtype=np.float32)
    E = np.asarray(inputs["E"], dtype=np.float32)
    Sw = np.asarray(inputs["Sw"], dtype=np.float32)
    Se = inputs["Se"]
    row_ind = int(inputs["row_ind"])
    word_i = np.asarray(inputs["word_i_indices"], dtype=np.int64)
    entity_j = np.asarray(inputs["entity_j_indices"], dtype=np.int64)
    sample_j = np.asarray(inputs["sample_j_indices"], dtype=np.int64)

    a8 = actual.astype(fp8)     # [N_E, N_W]
    p8 = prediction.astype(fp8)
    E8 = E.astype(fp8)
    W8 = W.astype(fp8)

    ident = np.zeros((128, 256), dtype=fp8)
    ident[:, 0:128] = np.eye(128)
    ident[:, 128:256] = -np.eye(128)

    ej_h = E8[entity_j].reshape(JB, 128, K).transpose(1, 0, 2).reshape(128, JB * K)
    ei_h = np.tile(E8[row_ind], (128, JB))
    sev_h = np.asarray(Se[row_ind])[entity_j].reshape(JB, 128).T.astype(np.float32)

    in_maps = []
    for c in range(NC):
        gsl = slice(c * GS, (c + 1) * GS)
        idx = word_i[gsl]                       # [GS, M]
        sj = sample_j[gsl]                      # [GS]

        # x: [NCHUNK, 128, 2, CCOL]; chunk c4 = (row-tile rt, col-range cc)
        x = np.empty((NCHUNK, 128, 2, CCOL), dtype=fp8)
        arows = a8[c * RS:(c + 1) * RS].reshape(NRT, 128, N_W)
        prows = p8[c * RS:(c + 1) * RS].reshape(NRT, 128, N_W)
        for ch in range(NCHUNK):
            rt, cc = ch // 4, ch % 4
            x[ch, :, 0, :] = arows[rt, :, cc * CCOL:(cc + 1) * CCOL]
            x[ch, :, 1, :] = prows[rt, :, cc * CCOL:(cc + 1) * CCOL]

        wi_h = np.ascontiguousarray(
            W8[:, idx].transpose(1, 0, 2).reshape(GS, K * M))

        smh = np.empty((128, SMH_TOT), dtype=fp8)
        smh[:, H_WSH:H_WSH + WSH] = W8[:, c * WSH:(c + 1) * WSH]
        smh[:, H_ESH:H_ESH + ESH] = (
            E8[c * RS:(c + 1) * RS].reshape(NRT, 128, K)
            .transpose(1, 0, 2).reshape(128, NRT * K))
        smh[:, H_EJ:H_EJ + JB * K] = ej_h
        smh[:, H_EI:H_EI + JB * K] = ei_h

        smf = np.empty((128, SMF_TOT), dtype=np.float32)
        smf[:, F_SWG:F_SWG + M] = Sw[sj[:, None], idx]
        smf[:, F_SEV:F_SEV + JB] = sev_h
        smf[:, F_WJ:F_WJ + K] = W[:, sj].T

        in_maps.append({
            "x": x, "wi": wi_h, "ident": ident, "smh": smh, "smf": smf,
        })
    return in_maps


def kernel(**inputs):
    global LAST_RESULTS
    import os

    if "nc" not in _CACHE:
        _CACHE["nc"] = _build_module()
    nc = _CACHE["nc"]

    in_maps = _shard_inputs(inputs)
    trace = bool(int(os.environ.get("KERNEL_TRACE", "0")))
    res = run_bass_kernel_spmd(nc, in_maps, list(range(NC)), trace=trace)
    LAST_RESULTS = res

    outs = [np.asarray(r["out"], dtype=np.float64) for r in res.results]

    recon2 = 0.0
    relw2 = 0.0
    rele2 = 0.0
    word = 0.0
    for c, o in enumerate(outs):
        # ACT stream tiles: accum col per tile
        for t in ACT_TILES:
            recon2 += o[:, O_ASQ + t].sum()
        # DVE stream tiles: bns 2x6 per tile
        for i, t in enumerate(DVE_TILES):
            b = o[:, O_BNS + i * 12:O_BNS + i * 12 + 12]
            recon2 += (b[:, 2] + b[:, 0] * b[:, 1] ** 2).sum()
            recon2 += (b[:, 5] + b[:, 3] * b[:, 4] ** 2).sum()
        relw2 += o[:, O_RELW].sum()
        rele2 += o[:, O_RELE].sum()
        dw2 = o[:, O_WI:O_WI + WIC].sum(axis=1)          # [128] per-group
        word += (np.sqrt(np.maximum(dw2, 0.0)) * o[:, O_SWS]).sum()

    ent = np.sqrt(outs[0][:, O_ENT].sum()) * outs[0][:, O_SEVS].sum()
    lamb = float(np.asarray(inputs["lamb"]))
    total = (np.sqrt(recon2) + lamb * (np.sqrt(relw2) + np.sqrt(rele2))
             + word + ent)
    return np.asarray(total, dtype=np.float32)


# revision 5
# speedup vs baseline: 1.9177x; 1.0128x over previous
"""Trainium2 Bass kernel for nn_CustomLoss (gnn_message_passing).

Computes, SPMD over 8 NeuronCores:
  loss = ||a - p||_F + lamb*(||relu(W)||_F + ||relu(E)||_F)
         + sum_g diff_w[g] * sum_m Sw[j_g, i_gm]
         + diff_e * sum(Se[row, e_j])

v2 design (vs the fp32-stream v1):
  - actual/prediction stream in fp8 (e4m3), planar pairs [128, 2, cols].
    The loss total (~4.2e6) is dominated by the word-similarity term;
    recon (~8e3) tolerates fp8 rounding (bias ~6e-4 on recon, ~1e-6 on
    the total).
  - The TensorE computes d = a - p via DoubleRow fp8 matmuls with a
    [+I; -I] stationary, writing full [128, 1024] psum tiles.
  - Stream squares: ACT (activation Square + accum, direct from psum)
    and DVE (bn_stats, 2x512 per tile; host recovers sum-of-squares
    from count/mean/M2). Tiles are assigned statically; each psum bank
    is read by exactly one engine (concurrent same-bank reads from two
    engines fault the HW).
  - Word term: wi gathered host-side (index routing), shipped fp8;
    Pool does dw = wj_bcast - wi (f32 out), ACT square-accums per
    group partition. Host does sqrt(dw2)*swsum.
  - All final reductions/sqrts on host from a [128, 240] partial
    tensor per core.
"""

import ml_dtypes
import numpy as np

import concourse.bass as bass
from concourse import mybir
from concourse.bass_utils import run_bass_kernel_spmd

NC = 8
N_E, N_W, K = 4096, 8192, 128
G, M, J = 1024, 64, 256
GS = G // NC              # 128 groups per core
RS = N_E // NC            # 512 rows of actual/prediction per core
NRT = RS // 128           # 4 row tiles per core

NCHUNK = 16               # x chunks per core
CCOL = 2048               # d-cols per chunk
NTILE = 32                # psum tiles (1024 cols each)
TCOL = 1024
NRING = 4                 # x ring slots
NPS = 4                   # psum slots

WIC = 8                   # wi processed in 8 chunks of 1024
WICOL = 1024
KPC = WICOL // M          # 16 k's per wi chunk

WSH = N_W // NC           # 1024 W columns per core (relu shard)
ESH = RS * K // 128       # 512 E values per partition (relu shard)
JB = J // 128             # 2 entity blocks

# smh (fp8) layout: wsh | esh | ej | ei
H_WSH = 0
H_ESH = H_WSH + WSH
H_EJ = H_ESH + ESH
H_EI = H_EJ + JB * K
SMH_TOT = H_EI + JB * K
# smf (f32) layout: swg | sev | wj
F_SWG = 0
F_SEV = F_SWG + M
F_WJ = F_SEV + JB
SMF_TOT = F_WJ + K

# outbuf (f32) column layout
O_ASQ = 0                  # 32: ACT stream-tile accums (only ACT tiles valid)
O_BNS = 32                 # 15*12: DVE bns stats, compact per DVE tile
O_WI = O_BNS + 15 * 12     # 8: wi-sq chunk accums (per-partition = per-group)
O_RELW = O_WI + WIC        # 1
O_RELE = O_RELW + 1        # 1
O_ENT = O_RELE + 1         # 1
O_SWS = O_ENT + 1          # 1
O_SEVS = O_SWS + 1         # 1
OUT_TOT = 240

f32 = mybir.dt.float32
f8 = mybir.dt.float8e4
bf16 = mybir.dt.bfloat16

SUB = mybir.AluOpType.subtract
MULT = mybir.AluOpType.mult
MAX = mybir.AluOpType.max
ADD = mybir.AluOpType.add
X = mybir.AxisListType.X
SQUARE = mybir.ActivationFunctionType.Square

# stream tile -> engine ('A' = ACT apsq, 'D' = DVE bn_stats)
TILE_ENG = ['D' if t % 5 in (1, 3) else 'A' for t in range(NTILE)]  # A19 D13
ACT_TILES = [t for t in range(NTILE) if TILE_ENG[t] == 'A']
DVE_TILES = [t for t in range(NTILE) if TILE_ENG[t] == 'D']
# positions in DVE's own stream-tile sequence after which to insert
# wi-sq chunks 0..7 (pool chunk j ready ~ 9.5 + 2.05*(j+1) us)
DVE_WI_POS = {2: 0, 3: 1, 4: 2, 6: 3, 7: 4, 8: 5, 10: 6, 11: 7}
DVE_RELW_POS = 5
DVE_RELE_POS = 9
DVE_ENT_POS = 6
DVE_SUMS_POS = 1
ACT_ENT_POS = 11

_CACHE = {}
LAST_RESULTS = None


def _build_module():
    from contextlib import ExitStack

    nc = bass.Bass()

    x_d = nc.dram_tensor("x", [NCHUNK, 128, 2, CCOL], f8, kind="ExternalInput")
    wi_d = nc.dram_tensor("wi", [128, K * M], f8, kind="ExternalInput")
    id_d = nc.dram_tensor("ident", [128, 256], f8, kind="ExternalInput")
    smh_d = nc.dram_tensor("smh", [128, SMH_TOT], f8, kind="ExternalInput")
    smf_d = nc.dram_tensor("smf", [128, SMF_TOT], f32, kind="ExternalInput")
    out_d = nc.dram_tensor("out", [128, OUT_TOT], f32, kind="ExternalOutput")

    ctx = ExitStack()
    xb = [ctx.enter_context(nc.sbuf_tensor(f"xb{b}", [128, 2, CCOL], f8))
          for b in range(NRING)]
    wib = ctx.enter_context(nc.sbuf_tensor("wib", [128, K * M], f8))
    idb = ctx.enter_context(nc.sbuf_tensor("idb", [128, 256], f8))
    smhb = ctx.enter_context(nc.sbuf_tensor("smhb", [128, SMH_TOT], f8))
    smfb = ctx.enter_context(nc.sbuf_tensor("smfb", [128, SMF_TOT], f32))
    dwb = ctx.enter_context(nc.sbuf_tensor("dwb", [128, K * M], bf16))
    ob = ctx.enter_context(nc.sbuf_tensor("ob", [128, OUT_TOT], f32))
    ascr = ctx.enter_context(nc.sbuf_tensor("ascr", [128, TCOL], f32))
    vwscr = ctx.enter_context(nc.sbuf_tensor("vwscr", [128, WICOL], bf16))
    vscr = ctx.enter_context(nc.sbuf_tensor("vscr", [128, WSH], f32))
    detb = ctx.enter_context(nc.sbuf_tensor("detb", [128, JB * K], f32))
    escr = ctx.enter_context(nc.sbuf_tensor("escr", [128, JB * K], f32))
    ps = [ctx.enter_context(nc.psum_tensor(f"ps{i}", [128, TCOL], f32))
          for i in range(NPS)]

    # all input DMAs inc one semaphore; issue order defines thresholds
    # (per-DMA-engine FIFO makes cumulative thresholds safe)
    s_in = ctx.enter_context(nc.semaphore("s_in"))
    s_mm = ctx.enter_context(nc.semaphore("s_mm"))
    s_psf = [ctx.enter_context(nc.semaphore(f"s_psf{i}")) for i in range(NPS)]
    s_dw = ctx.enter_context(nc.semaphore("s_dw"))
    s_ent = ctx.enter_context(nc.semaphore("s_ent"))
    s_done = ctx.enter_context(nc.semaphore("s_done"))
    s_out = ctx.enter_context(nc.semaphore("s_out"))

    # DMA issue ordinals (1-based) for s_in thresholds
    ORD_ID, ORD_X0, ORD_X1, ORD_SMF, ORD_WI, ORD_X2, ORD_SMH, ORD_X3 = range(1, 9)

    def xord(c):
        return {0: ORD_X0, 1: ORD_X1, 2: ORD_X2, 3: ORD_X3}.get(c, 5 + c)

    def wj_bcast(j):
        # [128, KPC] slice of wj, broadcast M times along free (k outer, m inner)
        sl = smfb[:, F_WJ + j * KPC:F_WJ + (j + 1) * KPC]
        return bass.AP(tensor=sl.tensor, offset=sl.offset, ap=[*sl.ap, [0, M]])

    with ctx, nc.Block(no_gpsimd_drain=True) as block:

        @block.sync
        def _(sync):
            sync.dma_start(out=idb[:], in_=id_d[:, :]).then_inc(s_in, 16)
            sync.dma_start(out=xb[0][:], in_=x_d[0, :, :, :]).then_inc(s_in, 16)
            sync.dma_start(out=xb[1][:], in_=x_d[1, :, :, :]).then_inc(s_in, 16)
            sync.dma_start(out=smfb[:], in_=smf_d[:, :]).then_inc(s_in, 16)
            sync.dma_start(out=wib[:], in_=wi_d[:, :]).then_inc(s_in, 16)
            sync.dma_start(out=xb[2][:], in_=x_d[2, :, :, :]).then_inc(s_in, 16)
            sync.dma_start(out=smhb[:], in_=smh_d[:, :]).then_inc(s_in, 16)
            sync.dma_start(out=xb[3][:], in_=x_d[3, :, :, :]).then_inc(s_in, 16)
            for c in range(NRING, NCHUNK):
                # ring slot reusable once both tiles of chunk c-NRING are matmul'd
                sync.wait_ge(s_mm, 2 * (c - NRING) + 2)
                sync.dma_start(out=xb[c % NRING][:],
                               in_=x_d[c, :, :, :]).then_inc(s_in, 16)
            sync.wait_ge(s_done, 2)
            sync.dma_start(out=out_d[:, :], in_=ob[:]).then_inc(s_out, 16)
            sync.wait_ge(s_out, 16)

        @block.tensor
        def _(t):
            t.wait_ge(s_in, 16 * ORD_ID)
            lhsT = idb[:].rearrange("p (two m) -> p two m", two=2)
            for ti in range(NTILE):
                c, s, b = ti // 2, ti % NPS, (ti // 2) % NRING
                if ti % 2 == 0:
                    t.wait_ge(s_in, 16 * xord(c))
                if ti >= NPS:
                    t.wait_ge(s_psf[s], ti // NPS)
                for h in range(2):
                    col = (ti % 2) * TCOL + h * 512
                    mm = nc.tensor.matmul(
                        out=ps[s][:, h * 512:(h + 1) * 512],
                        lhsT=lhsT,
                        rhs=xb[b][:, :, col:col + 512],
                        start=True, stop=True,
                        perf_mode=mybir.MatmulPerfMode.DoubleRow,
                    )
                    if h == 1:
                        mm.then_inc(s_mm, 1)

        @block.scalar
        def _(a):
            # warm the activation-function table before data arrives
            a.activation(out=ascr[:, 0:1], in_=ascr[:, 0:1], func=SQUARE)
            for i, ti in enumerate(ACT_TILES):
                s = ti % NPS
                a.wait_ge(s_mm, ti + 1)
                a.activation(out=ascr[:], in_=ps[s][:],
                             func=SQUARE,
                             accum_out=ob[:, O_ASQ + ti:O_ASQ + ti + 1]
                             ).then_inc(s_psf[s], 1)
                if i == ACT_ENT_POS:
                    a.wait_ge(s_ent, 1)
                    a.activation(out=escr[:], in_=detb[:], func=SQUARE,
                                 accum_out=ob[:, O_ENT:O_ENT + 1])
            a.copy(ob[:, O_ASQ:O_ASQ + 1], ob[:, O_ASQ:O_ASQ + 1]).then_inc(s_done, 1)

        @block.vector
        def _(v):
            for i, ti in enumerate(DVE_TILES):
                s = ti % NPS
                v.wait_ge(s_mm, ti + 1)
                v.bn_stats(out=ob[:, O_BNS + i * 12:O_BNS + i * 12 + 6],
                           in_=ps[s][:, 0:512])
                v.bn_stats(out=ob[:, O_BNS + i * 12 + 6:O_BNS + i * 12 + 12],
                           in_=ps[s][:, 512:TCOL]).then_inc(s_psf[s], 1)
                if i == DVE_SUMS_POS:
                    v.wait_ge(s_in, 16 * ORD_SMF)
                    v.reduce_sum(ob[:, O_SWS:O_SWS + 1],
                                 smfb[:, F_SWG:F_SWG + M], axis=X)
                    v.reduce_sum(ob[:, O_SEVS:O_SEVS + 1],
                                 smfb[:, F_SEV:F_SEV + JB], axis=X)
                if i == DVE_RELW_POS:
                    v.wait_ge(s_in, 16 * ORD_SMH)
                    v.scalar_tensor_tensor(
                        out=vscr[:, 0:WSH], in0=smhb[:, H_WSH:H_WSH + WSH],
                        scalar=0.0, in1=smhb[:, H_WSH:H_WSH + WSH],
                        op0=MAX, op1=MULT,
                        accum_out=ob[:, O_RELW:O_RELW + 1])
                if i == DVE_RELE_POS:
                    v.scalar_tensor_tensor(
                        out=vscr[:, 0:ESH], in0=smhb[:, H_ESH:H_ESH + ESH],
                        scalar=0.0, in1=smhb[:, H_ESH:H_ESH + ESH],
                        op0=MAX, op1=MULT,
                        accum_out=ob[:, O_RELE:O_RELE + 1])
                if i == DVE_ENT_POS:
                    v.tensor_tensor(out=detb[:],
                                    in0=smhb[:, H_EJ:H_EJ + JB * K],
                                    in1=smhb[:, H_EI:H_EI + JB * K],
                                    op=SUB).then_inc(s_ent, 1)
                if i in DVE_WI_POS:
                    j = DVE_WI_POS[i]
                    v.wait_ge(s_dw, j + 1)
                    v.scalar_tensor_tensor(
                        out=vwscr[:],
                        in0=dwb[:, j * WICOL:(j + 1) * WICOL], scalar=0.0,
                        in1=dwb[:, j * WICOL:(j + 1) * WICOL],
                        op0=mybir.AluOpType.bypass, op1=MULT,
                        accum_out=ob[:, O_WI + j:O_WI + j + 1])
            v.engine_nop().then_inc(s_done, 1)

        @block.gpsimd
        def _(g):
            g.wait_ge(s_in, 16 * ORD_WI)
            for j in range(WIC):
                g.tensor_tensor(
                    out=dwb[:, j * WICOL:(j + 1) * WICOL],
                    in0=wj_bcast(j),
                    in1=wib[:, j * WICOL:(j + 1) * WICOL],
                    op=SUB).then_inc(s_dw, 1)

    return nc


def _shard_inputs(inputs):
    fp8 = ml_dtypes.float8_e4m3
    actual = np.asarray(inputs["actual"], dtype=np.float32)
    prediction = np.asarray(inputs["prediction"], dtype=np.float32)
    W = np.asarray(inputs["W"], dtype=np.float32)
    E = np.asarray(inputs["E"], dtype=np.float32)
    Sw = np.asarray(inputs["Sw"], dtype=np.float32)
    Se = inputs["Se"]
    row_ind = int(inputs["row_ind"])
    word_i = np.asarray(inputs["word_i_indices"], dtype=np.int64)
    entity_j = np.asarray(inputs["entity_j_indices"], dtype=np.int64)
    sample_j = np.asarray(inputs["sample_j_indices"], dtype=np.int64)

    a8 = actual.astype(fp8)     # [N_E, N_W]
    p8 = prediction.astype(fp8)
    E8 = E.astype(fp8)
    W8 = W.astype(fp8)

    ident = np.zeros((128, 256), dtype=fp8)
    ident[:, 0:128] = np.eye(128)
    ident[:, 128:256] = -np.eye(128)

    ej_h = E8[entity_j].reshape(JB, 128, K).transpose(1, 0, 2).reshape(128, JB * K)
    ei_h = np.tile(E8[row_ind], (128, JB))
    sev_h = np.asarray(Se[row_ind])[entity_j].reshape(JB, 128).T.astype(np.float32)

    in_maps = []
    for c in range(NC):
        gsl = slice(c * GS, (c + 1) * GS)
        idx = word_i[gsl]                       # [GS, M]
        sj = sample_j[gsl]                      # [GS]

        # x: [NCHUNK, 128, 2, CCOL]; chunk c4 = (row-tile rt, col-range cc)
        x = np.empty((NCHUNK, 128, 2, CCOL), dtype=fp8)
        arows = a8[c * RS:(c + 1) * RS].reshape(NRT, 128, N_W)
        prows = p8[c * RS:(c + 1) * RS].reshape(NRT, 128, N_W)
        for ch in range(NCHUNK):
            rt, cc = ch // 4, ch % 4
            x[ch, :, 0, :] = arows[rt, :, cc * CCOL:(cc + 1) * CCOL]
            x[ch, :, 1, :] = prows[rt, :, cc * CCOL:(cc + 1) * CCOL]

        wi_h = np.ascontiguousarray(
            W8[:, idx].transpose(1, 0, 2).reshape(GS, K * M))

        smh = np.empty((128, SMH_TOT), dtype=fp8)
        smh[:, H_WSH:H_WSH + WSH] = W8[:, c * WSH:(c + 1) * WSH]
        smh[:, H_ESH:H_ESH + ESH] = (
            E8[c * RS:(c + 1) * RS].reshape(NRT, 128, K)
            .transpose(1, 0, 2).reshape(128, NRT * K))
        smh[:, H_EJ:H_EJ + JB * K] = ej_h
        smh[:, H_EI:H_EI + JB * K] = ei_h

        smf = np.empty((128, SMF_TOT), dtype=np.float32)
        smf[:, F_SWG:F_SWG + M] = Sw[sj[:, None], idx]
        smf[:, F_SEV:F_SEV + JB] = sev_h
        smf[:, F_WJ:F_WJ + K] = W[:, sj].T

        in_maps.append({
            "x": x, "wi": wi_h, "ident": ident, "smh": smh, "smf": smf,
        })
    return in_maps


def kernel(**inputs):
    global LAST_RESULTS
    import os

    if "nc" not in _CACHE:
        _CACHE["nc"] = _build_module()
    nc = _CACHE["nc"]

    in_maps = _shard_inputs(inputs)
    trace = bool(int(os.environ.get("KERNEL_TRACE", "0")))
    res = run_bass_kernel_spmd(nc, in_maps, list(range(NC)), trace=trace)
    LAST_RESULTS = res

    outs = [np.asarray(r["out"], dtype=np.float64) for r in res.results]

    recon2 = 0.0
    relw2 = 0.0
    rele2 = 0.0
    word = 0.0
    for c, o in enumerate(outs):
        # ACT stream tiles: accum col per tile
        for t in ACT_TILES:
            recon2 += o[:, O_ASQ + t].sum()
        # DVE stream tiles: bns 2x6 per tile
        for i, t in enumerate(DVE_TILES):
            b = o[:, O_BNS + i * 12:O_BNS + i * 12 + 12]
            recon2 += (b[:, 2] + b[:, 0] * b[:, 1] ** 2).sum()
            recon2 += (b[:, 5] + b[:, 3] * b[:, 4] ** 2).sum()
        relw2 += o[:, O_RELW].sum()
        rele2 += o[:, O_RELE].sum()
        dw2 = o[:, O_WI:O_WI + WIC].sum(axis=1)          # [128] per-group
        word += (np.sqrt(np.maximum(dw2, 0.0)) * o[:, O_SWS]).sum()

    ent = np.sqrt(outs[0][:, O_ENT].sum()) * outs[0][:, O_SEVS].sum()
    lamb = float(np.asarray(inputs["lamb"]))
    total = (np.sqrt(recon2) + lamb * (np.sqrt(relw2) + np.sqrt(rele2))
             + word + ent)
    return np.asarray(total, dtype=np.float32)
